# revision 22
# baseline (speedup 1.0000x reference)
"""Trainium2 Bass kernel for nn_DynamicComposeBlock.

Math (per (b,t)):
    out[o,h,w] = (sum_c W3d[o,c]*th[c,h]*tw[c,w] + b3d[o]) * (1-heat)*mask
                 + (sum_c W1d[o,c]*obj[c] + b1d[o]) * heat*mask

Key identity: with A = (1-heat)*mask and hm = heat*mask (functions of (h,w)
only), the blend commutes through the channel contraction:
    (W @ M) * A = W @ (M * A)        [M = th (x) tw outer product]
so the kernel computes M' = (th (x) tw) * A on the vector engine and a single
accumulated matmul  psum[o,hw] = W3dT.T @ M' + b3d (x) A + u (x) hm  on the
tensor engine, where u = W1d @ fea_obj + b1d (host-computed, tiny).

v3 layout:
  - f16 stores, host upcasts.
  - A_rep via partition-broadcast DMA from DRAM (no PE broadcast matmuls).
  - rank-1 term as a K=2 matmul (lxp=[2,O] rows b3d,u; rx=[2,HW] rows A,hm)
    -- no zero padding, no memsets.
  - t2-major matmul order so the PE never waits on the second M' half
    (PE p-state needs a gapless stream to hold 2.4 GHz).
  - evac split scalar(7/8) + vector(1/8); one mk-mul per bt on gpsimd;
    stores batched [128,2048] and issued on gpsimd; loads on sync.

Sharding: the 32 (b,t) pairs are split 4 per core across 8 cores; the small
weights are replicated. Each core writes its disjoint [4, 256, 64*64] slice.
"""
import os
import sys

for _p in ("/opt/trn_rl_repo",):
    if _p not in sys.path:
        sys.path.insert(0, _p)

import numpy as np

import concourse.bass as bass
import concourse.tile as tile
from concourse import bacc, mybir
from concourse.bass_utils import run_bass_kernel_spmd

N_CORES = 8
B, C, O, T, H, W = 2, 256, 256, 16, 64, 64
HW = H * W                      # 4096
JB = (B * T) // N_CORES         # 4 (b,t) pairs per core
KC = C // 128                   # 2 contraction chunks
OC = O // 128                   # 2 output-channel chunks

F32 = mybir.dt.float32
F16 = mybir.dt.float16

TRACE = {"on": False}  # test.py flips this to get HW exec time
USE_F16 = True


def build_nc():
    nc = bacc.Bacc("TRN2", target_bir_lowering=False, debug=False)

    def din(name, shape, dt=F16):
        return nc.dram_tensor(name, shape, dt, kind="ExternalInput").ap()

    th2_d = din("th2", [JB, C, H, 2])      # th duplicated in pairs
    tw_d = din("twf", [JB, C, W])
    w3_d = din("w3m", [C, O])              # W3d.T
    rows_d = din("rows", [JB, 2, HW])      # [A; hm] per (b,t)
    lx_d = din("lxp", [JB, 2, O])          # [b3d; u_j] per (b,t)
    ar_d = din("arep", [JB, 128, HW])      # A row pre-repeated x128 (host)
    out_d = nc.dram_tensor("out", [JB, O, HW], F16, kind="ExternalOutput").ap()

    with tile.TileContext(nc) as tc:
        with (
            tc.tile_pool(name="const", bufs=1) as pconst,
            tc.tile_pool(name="pin", bufs=3) as pin,
            tc.tile_pool(name="pam", bufs=2) as pam,
            tc.tile_pool(name="pm", bufs=3) as pm,
            tc.tile_pool(name="pmp", bufs=2) as pmp,
            tc.tile_pool(name="posb", bufs=3) as posb,
            tc.tile_pool(name="pso", bufs=4, space="PSUM") as pso,
        ):
            # rank-1 rhs tiles: rows 0-1 per (b,t), rows 2..127 stay zero
            # (contracted against lxp zero rows; must not be NaN garbage).
            # Only rows 2.. are memset so the rows 0-1 DMA needn't wait.
            rx0 = pconst.tile([128, HW], F16, tag="rx0")
            rx1 = pconst.tile([128, HW], F16, tag="rx1")
            rx2 = pconst.tile([128, HW], F16, tag="rx2")
            rx = [rx0, rx1, rx2]
            lxp = pconst.tile([128, JB, O], F16, tag="lxp")
            w3 = pconst.tile([128, KC, O], F16)
            wwarm = pconst.tile([128, 512], F16, tag="wwarm")
            nc.vector.memset(wwarm[:], 0.0)
            nc.vector.memset(lxp[:], 0.0)
            nc.gpsimd.memset(rx0[:], 0.0)
            nc.gpsimd.memset(rx1[:], 0.0)
            nc.gpsimd.memset(rx2[:], 0.0)

            areps = {}
            ths = {}
            tws = {}

            def prep(j, split_arep=False):
                """input loads for iteration j."""
                arep = pam.tile([128, HW], F16, tag="arep")
                rxj = rx[j % 3]
                th2 = pin.tile([128, KC, H, 2], F16, tag="th2")
                nc.sync.dma_start(
                    th2[:], th2_d[j].rearrange("(k p) h two -> p k h two", p=128)
                )
                ths[j] = th2
                twt = pin.tile([128, KC, W], F16, tag="twt")
                nc.sync.dma_start(
                    twt[:], tw_d[j].rearrange("(k p) w -> p k w", p=128)
                )
                tws[j] = twt
                if split_arep:
                    # ramp: arep halves so the first mp-mul gates on 512KB
                    # only; w3 lands just before the first matmul
                    hwh = HW // 2
                    nc.sync.dma_start(arep[:, 0:hwh], ar_d[j, :, 0:hwh])
                    nc.sync.dma_start(
                        w3[:], w3_d.rearrange("(k p) o -> p k o", p=128)
                    )
                    nc.sync.dma_start(arep[:, hwh:HW], ar_d[j, :, hwh:HW])
                else:
                    nc.sync.dma_start(arep[:], ar_d[j])
                nc.sync.dma_start(rxj[0:2, :], rows_d[j])
                nc.sync.dma_start(lxp[0:2, j, :], lx_d[j])
                areps[j] = arep

            prep(0, split_arep=True)
            # warm the PE p-state during the load ramp: ~14 throwaway
            # matmuls on a zeroed tile keep the PE busy >3us so the real
            # stream starts at the 2.4 GHz p-state
            warm = pso.tile([128, 1024], F32, tag="psq")
            for _ in range(13):
                nc.tensor.matmul(
                    warm[:, 0:512], wwarm[:, 0:128], wwarm[:],
                    start=True, stop=True,
                )
            for j in range(JB):
                if j + 1 < JB:
                    prep(j + 1)
                rxj = rx[j % 3]
                th2, twt, arep = ths[j], tws[j], areps[j]

                # ---- M' = (th (x) tw) * A, half-row granularity so the
                # out-matmuls on the first 2048 columns unblock early ----
                mp = pmp.tile([128, KC, HW], F16)
                HH = H // 2
                for half in range(2):
                    hs = slice(half * HH, (half + 1) * HH)
                    ns = slice(half * (HW // 2), (half + 1) * (HW // 2))
                    for k in range(KC):
                        mk = pm.tile([128, HW // 2], F16, tag="mk")
                        i0 = th2[:, k, hs].unsqueeze(2).broadcast_to(
                            [128, HH, W // 2, 2]
                        )
                        i1 = (
                            twt[:, k].unsqueeze(1).broadcast_to([128, HH, W])
                            .rearrange("p h (a b) -> p h a b", b=2)
                        )
                        mo = mk[:].rearrange("p (h a b) -> p h a b", h=HH, b=2)
                        nc.vector.tensor_mul(mo, i0, i1)
                        nc.vector.tensor_mul(mp[:, k, ns], mk[:], arep[:, ns])

                # ---- psum[o, hw] = W3dT.T @ M' + rank-1, t2-major ----
                osbs = [
                    posb.tile([128, HW], F16, tag=f"osb{oc}", name=f"osb{oc}")
                    for oc in range(OC)
                ]
                for t2 in range(HW // 1024):
                    for oc in range(OC):
                        osl = slice(oc * 128, oc * 128 + 128)
                        psq = pso.tile([128, 1024], F32)
                        nsls = [
                            slice(t2 * 1024 + hh * 512, t2 * 1024 + hh * 512 + 512)
                            for hh in range(2)
                        ]
                        psls = [psq[:, hh * 512 : hh * 512 + 512] for hh in range(2)]
                        for hh in range(2):
                            nc.tensor.matmul(
                                psls[hh], w3[:, 0, osl], mp[:, 0, nsls[hh]],
                                start=True, stop=False,
                            )
                        for hh in range(2):
                            nc.tensor.matmul(
                                psls[hh], w3[:, 1, osl], mp[:, 1, nsls[hh]],
                                start=False, stop=False,
                            )
                        for hh in range(2):
                            nc.tensor.matmul(
                                psls[hh], lxp[:, j, osl], rxj[:, nsls[hh]],
                                start=False, stop=True,
                            )
                        # evac: f32 psum -> f16 sbuf (scalar 7/8, vector 1/8;
                        # on the last (b,t) vector drains the final column)
                        ob = osbs[oc][:, t2 * 1024 : (t2 + 1) * 1024]
                        last = j == JB - 1
                        von = (t2 == 3 if last else t2 == 1) and oc == 1
                        if von:
                            nc.vector.tensor_copy(ob, psq[:])
                        else:
                            nc.scalar.copy(ob, psq[:])
                        if last and t2 == 3:
                            # final stores via HWDGE sync queue: cheap drain
                            osl_ = slice(oc * 128, oc * 128 + 128)
                            nc.sync.dma_start(
                                out_d[j, osl_, t2 * 1024 : (t2 + 1) * 1024],
                                ob,
                            )
                    # stores: [128, 2048] per oc at t2 boundaries 1 and 3
                    if t2 == 1 or (t2 == 3 and j < JB - 1):
                        cs = slice((t2 - 1) * 1024, (t2 + 1) * 1024)
                        for oc in range(OC):
                            osl = slice(oc * 128, oc * 128 + 128)
                            nc.gpsimd.dma_start(
                                out_d[j, osl, cs], osbs[oc][:, cs]
                            )
                    elif t2 == 2 and j == JB - 1:
                        for oc in range(OC):
                            osl = slice(oc * 128, oc * 128 + 128)
                            nc.sync.dma_start(
                                out_d[j, osl, 2048:3072],
                                osbs[oc][:, 2048:3072],
                            )

    nc.compile()
    return nc


_NC_CACHE = {}


def _get_nc():
    if "nc" not in _NC_CACHE:
        _NC_CACHE["nc"] = build_nc()
    return _NC_CACHE["nc"]


def kernel(fea_th, fea_tw, fea_obj, heatmap, mask, W3d, b3d, W1d, b1d):
    fea_th = np.asarray(fea_th, np.float32)
    fea_tw = np.asarray(fea_tw, np.float32)
    fea_obj = np.asarray(fea_obj, np.float32)
    heatmap = np.asarray(heatmap, np.float32)
    mask = np.asarray(mask, np.float32)
    W3d = np.asarray(W3d, np.float32)
    b3d = np.asarray(b3d, np.float32).reshape(O)
    b1d = np.asarray(b1d, np.float32).reshape(O)
    W1d = np.asarray(W1d, np.float32)
    w3m = np.ascontiguousarray(W3d.T).astype(np.float16)

    heat_f = heatmap[:, 0].reshape(B * T, HW)
    mask_f = mask[:, 0].reshape(B * T, HW)
    arow_f = ((1.0 - heat_f) * mask_f).astype(np.float16)
    hmrow_f = (heat_f * mask_f).astype(np.float16)
    # u[bt, o] = W1d @ fea_obj[bt] + b1d  (tiny; host-side)
    u_all = (
        np.einsum("oc,bct->bto", W1d, fea_obj, optimize=True)
        + b1d[None, None, :]
    ).reshape(B * T, O)

    nc = _get_nc()
    in_maps = []
    for core in range(N_CORES):
        bts = [divmod(core * JB + j, T) for j in range(JB)]
        bti = [b * T + t for b, t in bts]
        th = np.stack([fea_th[b, :, t, :] for b, t in bts])       # [JB, C, H]
        tw = np.stack([fea_tw[b, :, t, :] for b, t in bts])       # [JB, C, W]
        lxp = np.zeros((JB, 2, O), np.float16)
        for j, i in enumerate(bti):
            lxp[j, 0] = b3d.astype(np.float16)
            lxp[j, 1] = u_all[i].astype(np.float16)
        m = {
            "th2": np.ascontiguousarray(
                np.repeat(th.astype(np.float16)[..., None], 2, axis=-1)
            ),
            "twf": np.ascontiguousarray(tw.astype(np.float16)),
            "w3m": w3m,
            "rows": np.ascontiguousarray(
                np.stack([np.stack([arow_f[i], hmrow_f[i]]) for i in bti])
            ),
            "lxp": lxp,
            "arep": np.ascontiguousarray(
                np.broadcast_to(
                    arow_f[bti][:, None, :], (JB, 128, HW)
                )
            ),
        }
        in_maps.append(m)

    res = run_bass_kernel_spmd(
        nc, in_maps, core_ids=list(range(N_CORES)), trace=TRACE["on"]
    )
    if TRACE["on"]:
        TRACE["exec_time_ns"] = res.exec_time_ns
        TRACE["mean_exec_time_ns"] = res.mean_exec_time_ns
        TRACE["trace_path"] = (
            res.instructions_and_trace[1] if res.instructions_and_trace else None
        )

    out = np.empty((B, O, T, H, W), np.float32)
    for core in range(N_CORES):
        o = res.results[core]["out"]                               # [JB, O, HW]
        for j in range(JB):
            b, t = divmod(core * JB + j, T)
            out[b, :, t] = o[j].reshape(O, H, W).astype(np.float32)
    return out


# revision 26
# speedup vs baseline: 1.0158x; 1.0158x over previous
"""Trainium2 Bass kernel for nn_DynamicComposeBlock.

Math (per (b,t)):
    out[o,h,w] = (sum_c W3d[o,c]*th[c,h]*tw[c,w] + b3d[o]) * (1-heat)*mask
                 + (sum_c W1d[o,c]*obj[c] + b1d[o]) * heat*mask

Key identity: with A = (1-heat)*mask and hm = heat*mask (functions of (h,w)
only), the blend commutes through the channel contraction:
    (W @ M) * A = W @ (M * A)        [M = th (x) tw outer product]
so the kernel computes M' = (th (x) tw) * A on the vector engine and a single
accumulated matmul  psum[o,hw] = W3dT.T @ M' + b3d (x) A + u (x) hm  on the
tensor engine, where u = W1d @ fea_obj + b1d (host-computed, tiny).

v3 layout:
  - f16 stores, host upcasts.
  - A_rep via partition-broadcast DMA from DRAM (no PE broadcast matmuls).
  - rank-1 term as a K=2 matmul (lxp=[2,O] rows b3d,u; rx=[2,HW] rows A,hm)
    -- no zero padding, no memsets.
  - t2-major matmul order so the PE never waits on the second M' half
    (PE p-state needs a gapless stream to hold 2.4 GHz).
  - evac split scalar(7/8) + vector(1/8); one mk-mul per bt on gpsimd;
    stores batched [128,2048] and issued on gpsimd; loads on sync.

Sharding: the 32 (b,t) pairs are split 4 per core across 8 cores; the small
weights are replicated. Each core writes its disjoint [4, 256, 64*64] slice.
"""
import os
import sys

for _p in ("/opt/trn_rl_repo",):
    if _p not in sys.path:
        sys.path.insert(0, _p)

import numpy as np

import concourse.bass as bass
import concourse.tile as tile
from concourse import bacc, mybir
from concourse.bass_utils import run_bass_kernel_spmd

N_CORES = 8
B, C, O, T, H, W = 2, 256, 256, 16, 64, 64
HW = H * W                      # 4096
JB = (B * T) // N_CORES         # 4 (b,t) pairs per core
KC = C // 128                   # 2 contraction chunks
OC = O // 128                   # 2 output-channel chunks

F32 = mybir.dt.float32
F16 = mybir.dt.float16

TRACE = {"on": False}  # test.py flips this to get HW exec time
USE_F16 = True


def build_nc():
    nc = bacc.Bacc("TRN2", target_bir_lowering=False, debug=False)

    def din(name, shape, dt=F16):
        return nc.dram_tensor(name, shape, dt, kind="ExternalInput").ap()

    th2_d = din("th2", [JB, C, H, 2])      # th duplicated in pairs
    tw_d = din("twf", [JB, C, W])
    w3_d = din("w3m", [C, O])              # W3d.T
    rows_d = din("rows", [JB, 2, HW])      # [A; hm] per (b,t)
    lx_d = din("lxp", [JB, 2, O])          # [b3d; u_j] per (b,t)
    ar_d = din("arep", [JB, 128, HW])      # A row pre-repeated x128 (host)
    out_d = nc.dram_tensor("out", [JB, O, HW], F16, kind="ExternalOutput").ap()

    with tile.TileContext(nc) as tc:
        with (
            tc.tile_pool(name="const", bufs=1) as pconst,
            tc.tile_pool(name="pin", bufs=3) as pin,
            tc.tile_pool(name="pam", bufs=2) as pam,
            tc.tile_pool(name="pm", bufs=3) as pm,
            tc.tile_pool(name="pmp", bufs=2) as pmp,
            tc.tile_pool(name="posb", bufs=3) as posb,
            tc.tile_pool(name="pso", bufs=4, space="PSUM") as pso,
        ):
            # rank-1 rhs tiles: rows 0-1 per (b,t), rows 2..127 stay zero
            # (contracted against lxp zero rows; must not be NaN garbage).
            # Only rows 2.. are memset so the rows 0-1 DMA needn't wait.
            rx0 = pconst.tile([128, HW], F16, tag="rx0")
            rx1 = pconst.tile([128, HW], F16, tag="rx1")
            rx2 = pconst.tile([128, HW], F16, tag="rx2")
            rx = [rx0, rx1, rx2]
            lxp = pconst.tile([128, JB, O], F16, tag="lxp")
            w3 = pconst.tile([128, KC, O], F16)
            wwarm = pconst.tile([128, 512], F16, tag="wwarm")
            nc.vector.memset(wwarm[:], 0.0)
            nc.vector.memset(lxp[:], 0.0)
            # rx0 in column halves: the first rank-1 matmuls (cols<2048)
            # unblock ~1.8us earlier on the ramp
            nc.gpsimd.memset(rx0[:, 0 : HW // 2], 0.0)
            nc.gpsimd.memset(rx0[:, HW // 2 : HW], 0.0)
            nc.gpsimd.memset(rx1[:], 0.0)
            nc.gpsimd.memset(rx2[:], 0.0)

            areps = {}
            ths = {}
            tws = {}

            def prep(j, split_arep=False):
                """input loads for iteration j."""
                arep = pam.tile([128, HW], F16, tag="arep")
                rxj = rx[j % 3]
                th2 = pin.tile([128, KC, H, 2], F16, tag="th2")
                HH = H // 2
                if split_arep:
                    # j=0 ramp order: everything the first matmuls need, in
                    # consumption order, split fine so nothing over-waits
                    nc.sync.dma_start(
                        th2[:, :, 0:HH],
                        th2_d[j, :, 0:HH].rearrange(
                            "(k p) h two -> p k h two", p=128
                        ),
                    )
                else:
                    nc.sync.dma_start(
                        th2[:],
                        th2_d[j].rearrange("(k p) h two -> p k h two", p=128),
                    )
                ths[j] = th2
                twt = pin.tile([128, KC, W], F16, tag="twt")
                nc.sync.dma_start(
                    twt[:], tw_d[j].rearrange("(k p) w -> p k w", p=128)
                )
                tws[j] = twt
                if split_arep:
                    q = HW // 4
                    nc.sync.dma_start(arep[:, 0:q], ar_d[j, :, 0:q])
                    nc.sync.dma_start(
                        w3[:], w3_d.rearrange("(k p) o -> p k o", p=128)
                    )
                    nc.sync.dma_start(rxj[0:2, :], rows_d[j])
                    nc.sync.dma_start(lxp[0:2, j, :], lx_d[j])
                    nc.sync.dma_start(arep[:, q : 2 * q], ar_d[j, :, q : 2 * q])
                    nc.sync.dma_start(
                        th2[:, :, HH:H],
                        th2_d[j, :, HH:H].rearrange(
                            "(k p) h two -> p k h two", p=128
                        ),
                    )
                    nc.sync.dma_start(
                        arep[:, 2 * q : HW], ar_d[j, :, 2 * q : HW]
                    )
                else:
                    nc.sync.dma_start(arep[:], ar_d[j])
                    nc.sync.dma_start(rxj[0:2, :], rows_d[j])
                    nc.sync.dma_start(lxp[0:2, j, :], lx_d[j])
                areps[j] = arep

            prep(0, split_arep=True)
            # warm the PE p-state during the load ramp: ~14 throwaway
            # matmuls on a zeroed tile keep the PE busy >3us so the real
            # stream starts at the 2.4 GHz p-state
            warm = pso.tile([128, 1024], F32, tag="psq")
            for _ in range(10):
                nc.tensor.matmul(
                    warm[:, 0:512], wwarm[:, 0:128], wwarm[:],
                    start=True, stop=True,
                )
            for j in range(JB):
                if j + 1 < JB:
                    prep(j + 1)
                rxj = rx[j % 3]
                th2, twt, arep = ths[j], tws[j], areps[j]

                # ---- M' = (th (x) tw) * A, half-row granularity so the
                # out-matmuls on the first 2048 columns unblock early.
                # j=0 builds half0 k-major at quarter granularity so the
                # t2=0 chunks all unblock as early as possible. ----
                mp = pmp.tile([128, KC, HW], F16)
                HH = H // 2

                def mkmul(k, half):
                    hs = slice(half * HH, (half + 1) * HH)
                    mk = pm.tile([128, HW // 2], F16, tag="mk", name="mk")
                    i0 = th2[:, k, hs].unsqueeze(2).broadcast_to(
                        [128, HH, W // 2, 2]
                    )
                    i1 = (
                        twt[:, k].unsqueeze(1).broadcast_to([128, HH, W])
                        .rearrange("p h (a b) -> p h a b", b=2)
                    )
                    mo = mk[:].rearrange("p (h a b) -> p h a b", h=HH, b=2)
                    nc.vector.tensor_mul(mo, i0, i1)
                    return mk

                def mpmul(mk, k, half, quarter=None):
                    hwh = HW // 2
                    if quarter is None:
                        ns = slice(half * hwh, (half + 1) * hwh)
                        ms = slice(0, hwh)
                    else:
                        q = HW // 4
                        ns = slice(half * hwh + quarter * q,
                                   half * hwh + (quarter + 1) * q)
                        ms = slice(quarter * q, (quarter + 1) * q)
                    nc.vector.tensor_mul(mp[:, k, ns], mk[:, ms], arep[:, ns])

                if j == 0:
                    mk0 = mkmul(0, 0)
                    mk1 = mkmul(1, 0)
                    mpmul(mk0, 0, 0, 0)
                    mpmul(mk1, 1, 0, 0)
                    mpmul(mk0, 0, 0, 1)
                    mpmul(mk1, 1, 0, 1)
                    mk0 = mkmul(0, 1)
                    mk1 = mkmul(1, 1)
                    mpmul(mk0, 0, 1)
                    mpmul(mk1, 1, 1)
                else:
                    for half in range(2):
                        for k in range(KC):
                            mk = mkmul(k, half)
                            mpmul(mk, k, half)

                # ---- psum[o, hw] = W3dT.T @ M' + rank-1, t2-major ----
                osbs = [
                    posb.tile([128, HW], F16, tag=f"osb{oc}", name=f"osb{oc}")
                    for oc in range(OC)
                ]
                for t2 in range(HW // 1024):
                    for oc in range(OC):
                        osl = slice(oc * 128, oc * 128 + 128)
                        psq = pso.tile([128, 1024], F32)
                        nsls = [
                            slice(t2 * 1024 + hh * 512, t2 * 1024 + hh * 512 + 512)
                            for hh in range(2)
                        ]
                        psls = [psq[:, hh * 512 : hh * 512 + 512] for hh in range(2)]
                        # rank-1 first: on the ramp it only needs the tiny
                        # rows/lxp loads, so the real stream starts early
                        for hh in range(2):
                            nc.tensor.matmul(
                                psls[hh], lxp[:, j, osl], rxj[:, nsls[hh]],
                                start=True, stop=False,
                            )
                        for hh in range(2):
                            nc.tensor.matmul(
                                psls[hh], w3[:, 0, osl], mp[:, 0, nsls[hh]],
                                start=False, stop=False,
                            )
                        for hh in range(2):
                            nc.tensor.matmul(
                                psls[hh], w3[:, 1, osl], mp[:, 1, nsls[hh]],
                                start=False, stop=True,
                            )
                        # evac: f32 psum -> f16 sbuf (scalar 7/8, vector 1/8;
                        # on the last (b,t) vector drains the final column)
                        ob = osbs[oc][:, t2 * 1024 : (t2 + 1) * 1024]
                        last = j == JB - 1
                        von = (t2 == 3 if last else t2 == 1) and oc == 1
                        if von:
                            nc.vector.tensor_copy(ob, psq[:])
                        else:
                            nc.scalar.copy(ob, psq[:])
                        if last and t2 == 3:
                            # final stores via HWDGE sync queue: cheap drain
                            osl_ = slice(oc * 128, oc * 128 + 128)
                            nc.sync.dma_start(
                                out_d[j, osl_, t2 * 1024 : (t2 + 1) * 1024],
                                ob,
                            )
                    # stores: [128, 2048] per oc at t2 boundaries 1 and 3
                    if t2 == 1 or (t2 == 3 and j < JB - 1):
                        cs = slice((t2 - 1) * 1024, (t2 + 1) * 1024)
                        for oc in range(OC):
                            osl = slice(oc * 128, oc * 128 + 128)
                            nc.gpsimd.dma_start(
                                out_d[j, osl, cs], osbs[oc][:, cs]
                            )
                    elif t2 == 2 and j == JB - 1:
                        for oc in range(OC):
                            osl = slice(oc * 128, oc * 128 + 128)
                            nc.sync.dma_start(
                                out_d[j, osl, 2048:3072],
                                osbs[oc][:, 2048:3072],
                            )

    nc.compile()
    return nc


_NC_CACHE = {}


def _get_nc():
    if "nc" not in _NC_CACHE:
        _NC_CACHE["nc"] = build_nc()
    return _NC_CACHE["nc"]


def kernel(fea_th, fea_tw, fea_obj, heatmap, mask, W3d, b3d, W1d, b1d):
    fea_th = np.asarray(fea_th, np.float32)
    fea_tw = np.asarray(fea_tw, np.float32)
    fea_obj = np.asarray(fea_obj, np.float32)
    heatmap = np.asarray(heatmap, np.float32)
    mask = np.asarray(mask, np.float32)
    W3d = np.asarray(W3d, np.float32)
    b3d = np.asarray(b3d, np.float32).reshape(O)
    b1d = np.asarray(b1d, np.float32).reshape(O)
    W1d = np.asarray(W1d, np.float32)
    w3m = np.ascontiguousarray(W3d.T).astype(np.float16)

    heat_f = heatmap[:, 0].reshape(B * T, HW)
    mask_f = mask[:, 0].reshape(B * T, HW)
    arow_f = ((1.0 - heat_f) * mask_f).astype(np.float16)
    hmrow_f = (heat_f * mask_f).astype(np.float16)
    # u[bt, o] = W1d @ fea_obj[bt] + b1d  (tiny; host-side)
    u_all = (
        np.einsum("oc,bct->bto", W1d, fea_obj, optimize=True)
        + b1d[None, None, :]
    ).reshape(B * T, O)

    nc = _get_nc()
    in_maps = []
    for core in range(N_CORES):
        bts = [divmod(core * JB + j, T) for j in range(JB)]
        bti = [b * T + t for b, t in bts]
        th = np.stack([fea_th[b, :, t, :] for b, t in bts])       # [JB, C, H]
        tw = np.stack([fea_tw[b, :, t, :] for b, t in bts])       # [JB, C, W]
        lxp = np.zeros((JB, 2, O), np.float16)
        for j, i in enumerate(bti):
            lxp[j, 0] = b3d.astype(np.float16)
            lxp[j, 1] = u_all[i].astype(np.float16)
        m = {
            "th2": np.ascontiguousarray(
                np.repeat(th.astype(np.float16)[..., None], 2, axis=-1)
            ),
            "twf": np.ascontiguousarray(tw.astype(np.float16)),
            "w3m": w3m,
            "rows": np.ascontiguousarray(
                np.stack([np.stack([arow_f[i], hmrow_f[i]]) for i in bti])
            ),
            "lxp": lxp,
            "arep": np.ascontiguousarray(
                np.broadcast_to(
                    arow_f[bti][:, None, :], (JB, 128, HW)
                )
            ),
        }
        in_maps.append(m)

    res = run_bass_kernel_spmd(
        nc, in_maps, core_ids=list(range(N_CORES)), trace=TRACE["on"]
    )
    if TRACE["on"]:
        TRACE["exec_time_ns"] = res.exec_time_ns
        TRACE["mean_exec_time_ns"] = res.mean_exec_time_ns
        TRACE["trace_path"] = (
            res.instructions_and_trace[1] if res.instructions_and_trace else None
        )

    out = np.empty((B, O, T, H, W), np.float32)
    for core in range(N_CORES):
        o = res.results[core]["out"]                               # [JB, O, HW]
        for j in range(JB):
            b, t = divmod(core * JB + j, T)
            out[b, :, t] = o[j].reshape(O, H, W).astype(np.float32)
    return out
